# revision 27
# baseline (speedup 1.0000x reference)
"""Trainium2 Bass kernel for a 2-layer dense-adjacency GAT (nn_GAT_17824114278677).

Sharding: nodes (rows of the attention matrix) are sharded across the 8
NeuronCores, 512 rows per core; weights and node features are replicated.
Two SPMD launches (one per GAT layer) with a host-side gather of the layer-1
output in between.

Per-core dataflow: attention tiles are computed TRANSPOSED, [j=128
partitions, r=512 rows], so the aggregation att @ Wh maps directly onto the
PE (contraction over j on partitions) with zero on-chip transposes.

Key identity: softmax is invariant to a per-row scale, so divide the whole
row by exp(s_r) and fold exp(d_j) into the stationary Wh.  The remaining
per-element factor is

    p[j,r] = exp(leaky_relu(t) - t) * m[j,r]     (t = s_r + d_j)
           = max(1, exp(-0.8 t)) * m[j,r]

computed two ways, assigned per key-chunk to balance ScalarE vs VectorE:
  path A (VectorE only): q = (zb[r] * w[j]) max 1   (one fused tensor_scalar,
      zb = exp(-0.8 s) broadcast, w = exp(-0.8 d) per-partition);
      p = q * mask01                               (grouped tensor_tensor)
  path C (ScalarE + VectorE): the host ships in_c = madd - 0.8(s+d) with
      madd the 0/-1000 additive mask, so E = Exp(in_c) = q*m needs NO bias
      and groups over 8 chunks in ONE ScalarE op; then
      p = E max mask01 (grouped tensor_tensor) -- masked entries stay 0,
      unmasked become max(q, 1).

softmax denominators ride along as the exp(d) column in the stationary
operand; division + ELU happen on the host on the tiny per-head
[HID+1, 512] outputs.

Wh = x @ W (0.4%% of the FLOPs) plus the per-node attention vectors
f_src/f_dst are computed on the host in fp32 and shipped pre-rounded to
bf16; all on-device attention/aggregation math runs in bf16 with fp32 PSUM
accumulation.
"""

import os
import sys
import time
from contextlib import ExitStack

for _p in ("/opt/trn_rl_repo", "/root/.axon_site/_ro/trn_rl_repo"):
    if os.path.isdir(_p) and _p not in sys.path:
        sys.path.append(_p)

import numpy as np
import ml_dtypes

import bass_rust
import concourse.bass as bass
import concourse.tile as tile
from concourse import mybir
from concourse.bass_utils import run_bass_kernel_spmd

BF16 = ml_dtypes.bfloat16
F32 = mybir.dt.float32
BF = mybir.dt.bfloat16

N = 4096          # nodes
NCORES = 8
R = N // NCORES   # rows (queries) per core
CJ = N // 128     # 32 key chunks
FIN = 512         # input feature dim of both layers
NA = 8            # path-A chunks per head (chunks 0..NA-1); NC = CJ - NA
GRPA = 4          # A-path group size
GRPC = 8          # C-path group size (= mask slice size)
NMQ = 4           # mask/whx DMA slices
SLC = CJ // NMQ   # 8 chunks per slice

CORE_IDS = list(range(NCORES))

LAST_PERF = {}


# ---------------------------------------------------------------------------
# walrus workaround: it rejects instructions carrying >1 sync-wait command
# ("Too many sync wait commands").  Move excess waits onto preceding
# same-engine NoOps -- semantically identical (same-engine waits are totally
# ordered before the instruction).
def _split_excess_waits(nc, max_waits: int = 1) -> int:
    n_split = 0
    for fn in nc.m.functions:
        for bb in fn.blocks:
            insts = bb.instructions
            new_insts = []
            changed = False
            for ins in insts:
                si = ins.sync_info
                waits = list(si.on_wait) if si is not None else []
                if len(waits) > max_waits:
                    extra, keep = waits[:-max_waits], waits[-max_waits:]
                    for k in range(0, len(extra), max_waits):
                        chunk = extra[k : k + max_waits]
                        nop = bass_rust.InstNoOp(
                            name=f"{ins.name}-wsplit{k}", ins=[], outs=[]
                        )
                        nop.engine = ins.engine
                        nop.sync_info = mybir.SyncInfo(on_wait=chunk, on_update=[])
                        new_insts.append(nop)
                        n_split += 1
                    si.on_wait = keep
                    changed = True
                new_insts.append(ins)
            if changed:
                bb.instructions = new_insts
    return n_split


# ---------------------------------------------------------------------------
def _build_layer(H: int, HID: int, na: int = NA):
    """One GAT layer, per-core program.

    Inputs (per core):
      whxin  [128, CJ, H, WPH] bf16  Wh*exp(d) per head + exp(d) column
      maskM  [128, CJ, R]   bf16  multiplicative 0/1 adjacency, transposed
      zbB    [128, H, R]    bf16  exp(-0.8 f_src) of this core's rows (bcast)
      wcol   [128, H*CJ]    f32   [p, h*CJ+c] = exp(-0.8 f_dst[h, 128c+p])
      incM   [128, H, NCG, GRPC, R] bf16  madd - 0.8(s+d) for C-path chunks
    Output:
      agg    [H, HID+1, R]  f32   rows 0..HID-1: unnormalized att @ Wh
                                  (transposed); row HID: softmax denominator
    """
    WPH = HID + 2  # per-head stride in Whx: HID cols + exp(d) col + pad
    nc_chunks = CJ - na
    NCG = nc_chunks // GRPC  # C-path groups per head

    nc = bass.Bass("TRN2", debug=False, num_devices=NCORES)
    whxin = nc.dram_tensor("whxin", [128, CJ, H, WPH], BF, kind="ExternalInput")
    maskM = nc.dram_tensor("maskM", [128, CJ, R], BF, kind="ExternalInput")
    zbB = nc.dram_tensor("zbB", [128, H, R], BF, kind="ExternalInput")
    wcol = nc.dram_tensor("wcol", [128, H * CJ], F32, kind="ExternalInput")
    incM = nc.dram_tensor(
        "incM", [128, H, NCG, GRPC, R], BF, kind="ExternalInput"
    )
    agg = nc.dram_tensor("agg", [H, HID + 1, R], F32, kind="ExternalOutput")

    EXP = mybir.ActivationFunctionType.Exp
    MAX = mybir.AluOpType.max
    MUL = mybir.AluOpType.mult

    with tile.TileContext(nc) as tc, ExitStack() as ctx:
        cpool = ctx.enter_context(tc.tile_pool(name="const", bufs=1))
        ipool = ctx.enter_context(tc.tile_pool(name="inc", bufs=5))
        tpool = ctx.enter_context(tc.tile_pool(name="work", bufs=3))
        opool = ctx.enter_context(tc.tile_pool(name="out", bufs=2))
        paq = ctx.enter_context(tc.tile_pool(name="psa", bufs=3, space="PSUM"))

        # in_c prefetch: head 0's group DMAs go on the Activation hwdge
        # queue -- a second DMA stream running in parallel with the SP
        # queue's mask/whx while ScalarE is still idle.  Later heads stream
        # on SP in compute order (ipool bufs bound the run-ahead).
        inc_t = {}
        def fetch_inc(h, g, eng=None):
            t = ipool.tile([128, GRPC, R], BF, tag="inc")
            (eng or nc.sync).dma_start(t[:], incM[:, h, g])
            inc_t[(h, g)] = t

        mask_t = [
            cpool.tile([128, SLC, R], BF, tag=f"mask{m}", name=f"mask{m}")
            for m in range(NMQ)
        ]
        whx_t = [
            cpool.tile([128, SLC, H, WPH], BF, tag=f"whx{m}", name=f"whx{m}")
            for m in range(NMQ)
        ]
        hs = SLC // 2
        # heads 0+1 in_c on the parallel Activation queue (ScalarE is idle
        # during startup)
        for hh in range(min(2, H)):
            for g in range(NCG):
                fetch_inc(hh, g, eng=nc.scalar)
        # SP queue, ordered for the C0 chain (m1) and A0 (m0a/w/zb)
        nc.sync.dma_start(mask_t[0][:, 0:hs, :], maskM[:, 0:hs, :])
        nc.sync.dma_start(whx_t[0][:, 0:hs], whxin[:, 0:hs])
        w_t = cpool.tile([128, H * CJ], F32, tag="wcol")
        nc.sync.dma_start(w_t[:], wcol[:])
        zb_t = cpool.tile([128, H, R], BF, tag="zb")
        nc.sync.dma_start(zb_t[:], zbB[:])
        nc.sync.dma_start(mask_t[1][:], maskM[:, SLC : 2 * SLC])
        nc.sync.dma_start(whx_t[1][:], whxin[:, SLC : 2 * SLC])
        nc.sync.dma_start(mask_t[0][:, hs:SLC, :], maskM[:, hs:SLC, :])
        nc.sync.dma_start(whx_t[0][:, hs:SLC], whxin[:, hs:SLC])
        for mq in range(2, NMQ):
            cs = slice(mq * SLC, (mq + 1) * SLC)
            nc.sync.dma_start(mask_t[mq][:], maskM[:, cs, :])
            nc.sync.dma_start(whx_t[mq][:], whxin[:, cs])

        def mask_ap(c0, G):  # [128, G, R] view; groups never straddle slices
            return mask_t[c0 // SLC][:, c0 % SLC : c0 % SLC + G, :]

        def whx_ap(c, h):  # [128, HID+1] stationary for chunk c, head h
            return whx_t[c // SLC][:, c % SLC, h, 0 : HID + 1]

        # per-head chunk-group schedule, C and A groups interleaved; C first
        # so the PSUM accumulation (and PE) starts on the earliest-ready data
        agrps = [(c, min(GRPA, na - c), "a") for c in range(0, na, GRPA)]
        cgrps = [(na + g * GRPC, GRPC, "c") for g in range(NCG)]
        groups = []
        for i in range(max(len(agrps), len(cgrps))):
            if i < len(cgrps):
                groups.append(cgrps[i])
            if i < len(agrps):
                groups.append(agrps[i])

        # ---- attention + aggregation --------------------------------------
        # the PSUM->SBUF copy of head h is emitted AFTER head h+1's groups:
        # it depends on head h's last matmul, and emitting it eagerly would
        # head-of-line-block the next head's activations on the in-order
        # ScalarE queue while PE drains.
        def flush_prev(prev):
            ph, ppa = prev
            o = opool.tile([HID + 1, R], F32, tag="aggo", name="aggo")
            nc.scalar.copy(o[:], ppa[:])
            nc.sync.dma_start(agg[ph], o[:])

        prev = None
        for h in range(H):
            pa = paq.tile([HID + 1, R], F32, tag="psa")
            for gi, (c0, G, kind) in enumerate(groups):
                if kind == "a":
                    # path A: q = (zb * w_j) max 1 ; p = q * m01   (all V)
                    p3p = tpool.tile([128, GRPA, R], BF, tag="p3a")
                    qp = tpool.tile([128, GRPA, R], BF, tag="qa")
                    for k in range(G):
                        o_ix = h * CJ + c0 + k
                        nc.vector.tensor_scalar(
                            qp[:, k, :], zb_t[:, h, :],
                            w_t[:, o_ix : o_ix + 1], 1.0, op0=MUL, op1=MAX,
                        )
                    nc.vector.tensor_tensor(
                        p3p[:, 0:G, :], qp[:, 0:G, :], mask_ap(c0, G), op=MUL,
                    )
                else:
                    # path C: E = exp(in_c) = q*m (grouped ScalarE op),
                    # p = E max m01 (grouped V op); emitted in half-groups
                    # of 4 so TT/matmuls pipeline behind the long Act
                    g = (c0 - na) // GRPC
                    p3p = tpool.tile([128, GRPC, R], BF, tag="p3c")
                    qp = tpool.tile([128, GRPC, R], BF, tag="qc")
                    it = inc_t.pop((h, g))
                    for half in (slice(0, G // 2), slice(G // 2, G)):
                        nc.scalar.activation(qp[:, half, :], it[:, half, :], EXP)
                        nc.vector.tensor_tensor(
                            p3p[:, half, :], qp[:, half, :],
                            mask_ap(c0 + half.start, G // 2), op=MAX,
                        )
                        for k in range(half.start, half.stop):
                            nc.tensor.matmul(
                                pa[:], whx_ap(c0 + k, h), p3p[:, k, :],
                                start=(gi == 0 and k == 0),
                                stop=(gi == len(groups) - 1 and k == G - 1),
                            )
                    if 2 <= h + 1 < H:
                        fetch_inc(h + 1, g)
                    continue
                for k in range(G):
                    nc.tensor.matmul(
                        pa[:], whx_ap(c0 + k, h), p3p[:, k, :],
                        start=(gi == 0 and k == 0),
                        stop=(gi == len(groups) - 1 and k == G - 1),
                    )
            if prev is not None:
                flush_prev(prev)
            prev = (h, pa)
        flush_prev(prev)

    return nc


_PROGS = {}


def _get_prog(H, HID, na=NA):
    """Build (and cache) the layer program with the walrus wait-split fix
    applied.  The fix is HW-only: CoreSim's event loop rejects the injected
    NoOps, so sim users should call _build_layer directly."""
    key = (H, HID, na)
    if key not in _PROGS:
        nc = _build_layer(H, HID, na)
        _split_excess_waits(nc)
        _PROGS[key] = nc
    return _PROGS[key]


def _elu(v):
    return np.where(v > 0, v, np.expm1(np.minimum(v, 0.0))).astype(np.float32)


def _host_inputs(f_src, f_dst, adj, Wh, H, na=NA):
    """Shared per-layer host prep.  f_src/f_dst [N, H] f32, adj [N, N] i32,
    Wh [N, H*HID] f32 (pre-activation per-head features)."""
    HID = Wh.shape[1] // H
    WPH = HID + 2
    NCG = (CJ - na) // GRPC
    fdst_arr = np.ascontiguousarray(
        f_dst.T.reshape(H, CJ, 128).transpose(2, 0, 1).reshape(128, H * CJ)
    ).astype(np.float32)
    w_arr = np.exp(-0.8 * fdst_arr).astype(np.float32)   # exp(-0.8 f_dst)

    # exp(f_dst) folded into the stationary operand; ones-col becomes exp(d)
    ev = np.exp(f_dst).astype(np.float32)  # [N, H]
    whx = np.zeros((128, CJ, H, WPH), np.float32)
    whx[:, :, :, :HID] = (
        (Wh.reshape(N, H, HID) * ev[:, :, None])
        .reshape(CJ, 128, H, HID).transpose(1, 0, 2, 3)
    )
    whx[:, :, :, HID] = ev.reshape(CJ, 128, H).transpose(1, 0, 2)

    # -0.8*f_dst laid out [128, c, h] for the C-path combined tiles
    fd8 = (
        (-0.8 * f_dst).T.reshape(H, CJ, 128).transpose(2, 1, 0)
    ).astype(np.float32)  # [128, CJ, H]

    shared = {
        "wcol": w_arr,
        "whxin": whx.astype(BF16),
    }
    per_core = []
    for i in range(NCORES):
        rows = slice(R * i, R * (i + 1))
        adjT = adj[rows, :].T.astype(np.float32)  # [N, R] 0/1
        madd = np.ascontiguousarray(
            ((adjT - 1.0) * 1000.0).reshape(CJ, 128, R).transpose(1, 0, 2)
        )  # [128, CJ, R] additive 0/-1000
        fs = np.ascontiguousarray(f_src[rows, :].T)  # [H, R]
        zln = -0.8 * fs
        d = dict(shared)
        d["maskM"] = np.ascontiguousarray(
            adjT.reshape(CJ, 128, R).transpose(1, 0, 2)
        ).astype(BF16)
        d["zbB"] = np.broadcast_to(
            np.exp(zln)[None, :, :], (128, H, R)
        ).astype(BF16)
        # in_c[p, h, g, k, r] = madd[p, na+g*GRPC+k, r] + zln[h, r] + fd8[p, c, h]
        cc = madd[:, na:, :].reshape(128, NCG, GRPC, 1, R)  # broadcast over h
        inc = (
            cc
            + zln[None, None, None, :, :]
            + fd8[:, na:, :].reshape(128, NCG, GRPC, H, 1)
        )  # [128, NCG, GRPC, H, R]
        d["incM"] = np.ascontiguousarray(
            inc.transpose(0, 3, 1, 2, 4)
        ).astype(BF16)
        per_core.append(d)
    return per_core


def _run_layer(nc, in_maps, H, HID, tag):
    t0 = time.time()
    res = run_bass_kernel_spmd(nc, in_maps, core_ids=CORE_IDS)
    LAST_PERF[f"{tag}_wall_s"] = time.time() - t0
    LAST_PERF[f"{tag}_exec_ns"] = res.exec_time_ns

    hT = np.empty((H * HID, N), np.float32)
    for i in range(NCORES):
        a = res.results[i]["agg"]  # [H, HID+1, R]
        denom = a[:, HID : HID + 1, :]
        hT[:, R * i : R * (i + 1)] = (a[:, :HID, :] / denom).reshape(H * HID, R)
    return hT


def kernel(x, adj, W1, a1, W2, a2):
    x = np.asarray(x, np.float32)
    adj = np.asarray(adj, np.int32)
    W1 = np.asarray(W1, np.float32)
    a1 = np.asarray(a1, np.float32)
    W2 = np.asarray(W2, np.float32)
    a2 = np.asarray(a2, np.float32)

    H1, HID1, OUT = W1.shape[0], W1.shape[2], W2.shape[1]

    progA = _get_prog(H1, HID1)
    progB = _get_prog(1, OUT)

    # ---- layer 1 ----------------------------------------------------------
    W1c = np.ascontiguousarray(W1.transpose(1, 0, 2).reshape(FIN, H1 * HID1))
    wsrc1 = np.einsum("hfk,hk->fh", W1, a1[:, :HID1, 0]).astype(np.float32)
    wdst1 = np.einsum("hfk,hk->fh", W1, a1[:, HID1:, 0]).astype(np.float32)
    f_src1 = x @ wsrc1  # [N, H]
    f_dst1 = x @ wdst1
    Wh1 = x @ W1c  # [N, H1*HID1]

    in_maps = _host_inputs(f_src1, f_dst1, adj, Wh1, H1)
    hT = _run_layer(progA, in_maps, H1, HID1, "layer1")
    hcatT = _elu(hT)  # [512, N] == h_cat.T (concat=True applies elu)

    # ---- layer 2 ----------------------------------------------------------
    hcat = np.ascontiguousarray(hcatT.T)  # [N, 512]
    wsrc2 = (W2 @ a2[:OUT, 0]).astype(np.float32)[:, None]
    wdst2 = (W2 @ a2[OUT:, 0]).astype(np.float32)[:, None]
    f_src2 = hcat @ wsrc2  # [N, 1]
    f_dst2 = hcat @ wdst2
    Wh2 = hcat @ W2  # [N, OUT]
    in_maps2 = _host_inputs(f_src2, f_dst2, adj, Wh2, 1)
    outT = _run_layer(progB, in_maps2, 1, OUT, "layer2")
    # layer 2: concat=False -> no inner elu; final output = elu(out)
    return np.ascontiguousarray(_elu(outT).T)


# revision 33
# speedup vs baseline: 1.0129x; 1.0129x over previous
"""Trainium2 Bass kernel for a 2-layer dense-adjacency GAT (nn_GAT_17824114278677).

Sharding: nodes (rows of the attention matrix) are sharded across the 8
NeuronCores, 512 rows per core; weights and node features are replicated.
Two SPMD launches (one per GAT layer) with a host-side gather of the layer-1
output in between.

Per-core dataflow: attention tiles are computed TRANSPOSED, [j=128
partitions, r=512 rows], so the aggregation att @ Wh maps directly onto the
PE (contraction over j on partitions) with zero on-chip transposes.

Key identity: softmax is invariant to a per-row scale, so divide the whole
row by exp(s_r) and fold exp(d_j) into the stationary Wh.  The remaining
per-element factor is

    p[j,r] = exp(leaky_relu(t) - t) * m[j,r]     (t = s_r + d_j)
           = max(1, exp(-0.8 t)) * m[j,r]

computed two ways, assigned per key-chunk to balance ScalarE vs VectorE:
  path A (VectorE only): q = (zb[r] * w[j]) max 1   (one fused tensor_scalar,
      zb = exp(-0.8 s) broadcast, w = exp(-0.8 d) per-partition);
      p = q * mask01                               (grouped tensor_tensor)
  path C (ScalarE + VectorE): the host ships in_c = madd - 0.8(s+d) with
      madd the 0/-1000 additive mask, so E = Exp(in_c) = q*m needs NO bias
      and groups over 8 chunks in ONE ScalarE op; then
      p = E max mask01 (grouped tensor_tensor) -- masked entries stay 0,
      unmasked become max(q, 1).

softmax denominators ride along as the exp(d) column in the stationary
operand; division + ELU happen on the host on the tiny per-head
[HID+1, 512] outputs.

Wh = x @ W (0.4%% of the FLOPs) plus the per-node attention vectors
f_src/f_dst are computed on the host in fp32 and shipped pre-rounded to
bf16; all on-device attention/aggregation math runs in bf16 with fp32 PSUM
accumulation.
"""

import os
import sys
import time
from contextlib import ExitStack

for _p in ("/opt/trn_rl_repo", "/root/.axon_site/_ro/trn_rl_repo"):
    if os.path.isdir(_p) and _p not in sys.path:
        sys.path.append(_p)

import numpy as np
import ml_dtypes

import bass_rust
import concourse.bass as bass
import concourse.tile as tile
from concourse import mybir
from concourse.bass_utils import run_bass_kernel_spmd

BF16 = ml_dtypes.bfloat16
F32 = mybir.dt.float32
BF = mybir.dt.bfloat16

N = 4096          # nodes
NCORES = 8
R = N // NCORES   # rows (queries) per core
CJ = N // 128     # 32 key chunks
FIN = 512         # input feature dim of both layers
NA = 8            # path-A chunks per head (chunks 0..NA-1); NC = CJ - NA
GRPA = 8          # A-path group size
GRPC = 8          # C-path group size (= mask slice size)
NMQ = 4           # mask/whx DMA slices
SLC = CJ // NMQ   # 8 chunks per slice

CORE_IDS = list(range(NCORES))

LAST_PERF = {}


# ---------------------------------------------------------------------------
# walrus workaround: it rejects instructions carrying >1 sync-wait command
# ("Too many sync wait commands").  Move excess waits onto preceding
# same-engine NoOps -- semantically identical (same-engine waits are totally
# ordered before the instruction).
def _split_excess_waits(nc, max_waits: int = 1) -> int:
    n_split = 0
    for fn in nc.m.functions:
        for bb in fn.blocks:
            insts = bb.instructions
            new_insts = []
            changed = False
            for ins in insts:
                si = ins.sync_info
                waits = list(si.on_wait) if si is not None else []
                if len(waits) > max_waits:
                    extra, keep = waits[:-max_waits], waits[-max_waits:]
                    for k in range(0, len(extra), max_waits):
                        chunk = extra[k : k + max_waits]
                        nop = bass_rust.InstNoOp(
                            name=f"{ins.name}-wsplit{k}", ins=[], outs=[]
                        )
                        nop.engine = ins.engine
                        nop.sync_info = mybir.SyncInfo(on_wait=chunk, on_update=[])
                        new_insts.append(nop)
                        n_split += 1
                    si.on_wait = keep
                    changed = True
                new_insts.append(ins)
            if changed:
                bb.instructions = new_insts
    return n_split


# ---------------------------------------------------------------------------
def _build_layer(H: int, HID: int, na: int = NA):
    """One GAT layer, per-core program.

    Inputs (per core):
      whxin  [128, CJ, H, WPH] bf16  Wh*exp(d) per head + exp(d) column
      maskM  [128, CJ, R]   bf16  multiplicative 0/1 adjacency, transposed
      zbB    [128, H, R]    bf16  exp(-0.8 f_src) of this core's rows (bcast)
      wcol   [128, H*CJ]    f32   [p, h*CJ+c] = exp(-0.8 f_dst[h, 128c+p])
      incM   [128, H, NCG, GRPC, R] bf16  madd - 0.8(s+d) for C-path chunks
    Output:
      agg    [H, HID+1, R]  f32   rows 0..HID-1: unnormalized att @ Wh
                                  (transposed); row HID: softmax denominator
    """
    WPH = HID + 2  # per-head stride in Whx: HID cols + exp(d) col + pad
    nc_chunks = CJ - na
    NCG = nc_chunks // GRPC  # C-path groups per head

    nc = bass.Bass("TRN2", debug=False, num_devices=NCORES)
    whxin = nc.dram_tensor("whxin", [128, CJ, H, WPH], BF, kind="ExternalInput")
    maskM = nc.dram_tensor("maskM", [128, CJ, R], BF, kind="ExternalInput")
    zbB = nc.dram_tensor("zbB", [128, H, R], BF, kind="ExternalInput")
    wcol = nc.dram_tensor("wcol", [128, H * CJ], F32, kind="ExternalInput")
    incM = nc.dram_tensor(
        "incM", [128, H, NCG, GRPC, R], BF, kind="ExternalInput"
    )
    agg = nc.dram_tensor("agg", [H, HID + 1, R], F32, kind="ExternalOutput")

    EXP = mybir.ActivationFunctionType.Exp
    MAX = mybir.AluOpType.max
    MUL = mybir.AluOpType.mult

    with tile.TileContext(nc) as tc, ExitStack() as ctx:
        cpool = ctx.enter_context(tc.tile_pool(name="const", bufs=1))
        ipool = ctx.enter_context(tc.tile_pool(name="inc", bufs=5))
        tpool = ctx.enter_context(tc.tile_pool(name="work", bufs=3))
        qpool = ctx.enter_context(tc.tile_pool(name="qwork", bufs=2))
        opool = ctx.enter_context(tc.tile_pool(name="out", bufs=2))
        paq = ctx.enter_context(tc.tile_pool(name="psa", bufs=3, space="PSUM"))

        # in_c prefetch: head 0's group DMAs go on the Activation hwdge
        # queue -- a second DMA stream running in parallel with the SP
        # queue's mask/whx while ScalarE is still idle.  Later heads stream
        # on SP in compute order (ipool bufs bound the run-ahead).
        inc_t = {}
        def fetch_inc(h, g, eng=None):
            t = ipool.tile([128, GRPC, R], BF, tag="inc")
            (eng or nc.sync).dma_start(t[:], incM[:, h, g])
            inc_t[(h, g)] = t

        mask_t = [
            cpool.tile([128, SLC, R], BF, tag=f"mask{m}", name=f"mask{m}")
            for m in range(NMQ)
        ]
        whx_t = [
            cpool.tile([128, SLC, H, WPH], BF, tag=f"whx{m}", name=f"whx{m}")
            for m in range(NMQ)
        ]
        hs = SLC // 2
        # heads 0+1 in_c on the parallel Activation queue (ScalarE is idle
        # during startup)
        for hh in range(min(2, H)):
            for g in range(NCG):
                fetch_inc(hh, g, eng=nc.scalar)
        # SP queue, ordered for the C0 chain (m1) and A0 (m0a/w/zb)
        nc.sync.dma_start(mask_t[0][:, 0:hs, :], maskM[:, 0:hs, :])
        nc.sync.dma_start(whx_t[0][:, 0:hs], whxin[:, 0:hs])
        w_t = cpool.tile([128, H * CJ], F32, tag="wcol")
        nc.sync.dma_start(w_t[:], wcol[:])
        zb_t = cpool.tile([128, H, R], BF, tag="zb")
        nc.sync.dma_start(zb_t[:], zbB[:])
        nc.sync.dma_start(mask_t[0][:, hs:SLC, :], maskM[:, hs:SLC, :])
        nc.sync.dma_start(whx_t[0][:, hs:SLC], whxin[:, hs:SLC])
        for mq in range(1, NMQ):
            cs = slice(mq * SLC, (mq + 1) * SLC)
            nc.sync.dma_start(mask_t[mq][:], maskM[:, cs, :])
            nc.sync.dma_start(whx_t[mq][:], whxin[:, cs])

        def mask_ap(c0, G):  # [128, G, R] view; groups never straddle slices
            return mask_t[c0 // SLC][:, c0 % SLC : c0 % SLC + G, :]

        def whx_ap(c, h):  # [128, HID+1] stationary for chunk c, head h
            return whx_t[c // SLC][:, c % SLC, h, 0 : HID + 1]

        # per-head chunk-group schedule: the A group (fed by the earliest
        # DMAs) first, then the C groups
        agrps = [(c, min(GRPA, na - c), "a") for c in range(0, na, GRPA)]
        cgrps = [(na + g * GRPC, GRPC, "c") for g in range(NCG)]
        groups = agrps + cgrps

        # ---- attention + aggregation --------------------------------------
        # the PSUM->SBUF copy of head h is emitted AFTER head h+1's groups:
        # it depends on head h's last matmul, and emitting it eagerly would
        # head-of-line-block the next head's activations on the in-order
        # ScalarE queue while PE drains.
        def flush_prev(prev):
            ph, ppa = prev
            o = opool.tile([HID + 1, R], F32, tag="aggo", name="aggo")
            nc.scalar.copy(o[:], ppa[:])
            nc.sync.dma_start(agg[ph], o[:])

        prev = None
        for h in range(H):
            pa = paq.tile([HID + 1, R], F32, tag="psa")
            for gi, (c0, G, kind) in enumerate(groups):
                if kind == "a":
                    # path A: q = (zb * w_j) max 1 ; p = q * m01   (all V)
                    p3p = tpool.tile([128, GRPA, R], BF, tag="p3a")
                    qp = qpool.tile([128, GRPA, R], BF, tag="qa")
                    for k in range(G):
                        o_ix = h * CJ + c0 + k
                        nc.vector.tensor_scalar(
                            qp[:, k, :], zb_t[:, h, :],
                            w_t[:, o_ix : o_ix + 1], 1.0, op0=MUL, op1=MAX,
                        )
                    nc.vector.tensor_tensor(
                        p3p[:, 0:G, :], qp[:, 0:G, :], mask_ap(c0, G), op=MUL,
                    )
                else:
                    # path C: E = exp(in_c) = q*m (one grouped ScalarE op),
                    # p = E max m01 (one grouped V op)
                    g = (c0 - na) // GRPC
                    p3p = tpool.tile([128, GRPC, R], BF, tag="p3c")
                    qp = qpool.tile([128, GRPC, R], BF, tag="qc")
                    it = inc_t.pop((h, g))
                    nc.scalar.activation(qp[:, 0:G, :], it[:], EXP)
                    if 2 <= h + 1 < H:
                        fetch_inc(h + 1, g)
                    nc.vector.tensor_tensor(
                        p3p[:, 0:G, :], qp[:, 0:G, :], mask_ap(c0, G), op=MAX,
                    )
                for k in range(G):
                    nc.tensor.matmul(
                        pa[:], whx_ap(c0 + k, h), p3p[:, k, :],
                        start=(gi == 0 and k == 0),
                        stop=(gi == len(groups) - 1 and k == G - 1),
                    )
            if prev is not None:
                flush_prev(prev)
            prev = (h, pa)
        flush_prev(prev)

    return nc


_PROGS = {}


def _get_prog(H, HID, na=NA):
    """Build (and cache) the layer program with the walrus wait-split fix
    applied.  The fix is HW-only: CoreSim's event loop rejects the injected
    NoOps, so sim users should call _build_layer directly."""
    key = (H, HID, na)
    if key not in _PROGS:
        nc = _build_layer(H, HID, na)
        _split_excess_waits(nc)
        _PROGS[key] = nc
    return _PROGS[key]


def _elu(v):
    return np.where(v > 0, v, np.expm1(np.minimum(v, 0.0))).astype(np.float32)


def _host_inputs(f_src, f_dst, adj, Wh, H, na=NA):
    """Shared per-layer host prep.  f_src/f_dst [N, H] f32, adj [N, N] i32,
    Wh [N, H*HID] f32 (pre-activation per-head features)."""
    HID = Wh.shape[1] // H
    WPH = HID + 2
    NCG = (CJ - na) // GRPC
    fdst_arr = np.ascontiguousarray(
        f_dst.T.reshape(H, CJ, 128).transpose(2, 0, 1).reshape(128, H * CJ)
    ).astype(np.float32)
    w_arr = np.exp(-0.8 * fdst_arr).astype(np.float32)   # exp(-0.8 f_dst)

    # exp(f_dst) folded into the stationary operand; ones-col becomes exp(d)
    ev = np.exp(f_dst).astype(np.float32)  # [N, H]
    whx = np.zeros((128, CJ, H, WPH), np.float32)
    whx[:, :, :, :HID] = (
        (Wh.reshape(N, H, HID) * ev[:, :, None])
        .reshape(CJ, 128, H, HID).transpose(1, 0, 2, 3)
    )
    whx[:, :, :, HID] = ev.reshape(CJ, 128, H).transpose(1, 0, 2)

    # -0.8*f_dst laid out [128, c, h] for the C-path combined tiles
    fd8 = (
        (-0.8 * f_dst).T.reshape(H, CJ, 128).transpose(2, 1, 0)
    ).astype(np.float32)  # [128, CJ, H]

    shared = {
        "wcol": w_arr,
        "whxin": whx.astype(BF16),
    }
    per_core = []
    for i in range(NCORES):
        rows = slice(R * i, R * (i + 1))
        adjT = adj[rows, :].T.astype(np.float32)  # [N, R] 0/1
        madd = np.ascontiguousarray(
            ((adjT - 1.0) * 1000.0).reshape(CJ, 128, R).transpose(1, 0, 2)
        )  # [128, CJ, R] additive 0/-1000
        fs = np.ascontiguousarray(f_src[rows, :].T)  # [H, R]
        zln = -0.8 * fs
        d = dict(shared)
        d["maskM"] = np.ascontiguousarray(
            adjT.reshape(CJ, 128, R).transpose(1, 0, 2)
        ).astype(BF16)
        d["zbB"] = np.broadcast_to(
            np.exp(zln)[None, :, :], (128, H, R)
        ).astype(BF16)
        # in_c[p, h, g, k, r] = madd[p, na+g*GRPC+k, r] + zln[h, r] + fd8[p, c, h]
        cc = madd[:, na:, :].reshape(128, NCG, GRPC, 1, R)  # broadcast over h
        inc = (
            cc
            + zln[None, None, None, :, :]
            + fd8[:, na:, :].reshape(128, NCG, GRPC, H, 1)
        )  # [128, NCG, GRPC, H, R]
        d["incM"] = np.ascontiguousarray(
            inc.transpose(0, 3, 1, 2, 4)
        ).astype(BF16)
        per_core.append(d)
    return per_core


def _run_layer(nc, in_maps, H, HID, tag):
    t0 = time.time()
    res = run_bass_kernel_spmd(nc, in_maps, core_ids=CORE_IDS)
    LAST_PERF[f"{tag}_wall_s"] = time.time() - t0
    LAST_PERF[f"{tag}_exec_ns"] = res.exec_time_ns

    hT = np.empty((H * HID, N), np.float32)
    for i in range(NCORES):
        a = res.results[i]["agg"]  # [H, HID+1, R]
        denom = a[:, HID : HID + 1, :]
        hT[:, R * i : R * (i + 1)] = (a[:, :HID, :] / denom).reshape(H * HID, R)
    return hT


def kernel(x, adj, W1, a1, W2, a2):
    x = np.asarray(x, np.float32)
    adj = np.asarray(adj, np.int32)
    W1 = np.asarray(W1, np.float32)
    a1 = np.asarray(a1, np.float32)
    W2 = np.asarray(W2, np.float32)
    a2 = np.asarray(a2, np.float32)

    H1, HID1, OUT = W1.shape[0], W1.shape[2], W2.shape[1]

    progA = _get_prog(H1, HID1)
    progB = _get_prog(1, OUT)

    # ---- layer 1 ----------------------------------------------------------
    W1c = np.ascontiguousarray(W1.transpose(1, 0, 2).reshape(FIN, H1 * HID1))
    wsrc1 = np.einsum("hfk,hk->fh", W1, a1[:, :HID1, 0]).astype(np.float32)
    wdst1 = np.einsum("hfk,hk->fh", W1, a1[:, HID1:, 0]).astype(np.float32)
    f_src1 = x @ wsrc1  # [N, H]
    f_dst1 = x @ wdst1
    Wh1 = x @ W1c  # [N, H1*HID1]

    in_maps = _host_inputs(f_src1, f_dst1, adj, Wh1, H1)
    hT = _run_layer(progA, in_maps, H1, HID1, "layer1")
    hcatT = _elu(hT)  # [512, N] == h_cat.T (concat=True applies elu)

    # ---- layer 2 ----------------------------------------------------------
    hcat = np.ascontiguousarray(hcatT.T)  # [N, 512]
    wsrc2 = (W2 @ a2[:OUT, 0]).astype(np.float32)[:, None]
    wdst2 = (W2 @ a2[OUT:, 0]).astype(np.float32)[:, None]
    f_src2 = hcat @ wsrc2  # [N, 1]
    f_dst2 = hcat @ wdst2
    Wh2 = hcat @ W2  # [N, OUT]
    in_maps2 = _host_inputs(f_src2, f_dst2, adj, Wh2, 1)
    outT = _run_layer(progB, in_maps2, 1, OUT, "layer2")
    # layer 2: concat=False -> no inner elu; final output = elu(out)
    return np.ascontiguousarray(_elu(outT).T)


# revision 34
# speedup vs baseline: 1.0374x; 1.0242x over previous
"""Trainium2 Bass kernel for a 2-layer dense-adjacency GAT (nn_GAT_17824114278677).

Sharding: nodes (rows of the attention matrix) are sharded across the 8
NeuronCores, 512 rows per core; weights and node features are replicated.
Two SPMD launches (one per GAT layer) with a host-side gather of the layer-1
output in between.

Per-core dataflow: attention tiles are computed TRANSPOSED, [j=128
partitions, r=512 rows], so the aggregation att @ Wh maps directly onto the
PE (contraction over j on partitions) with zero on-chip transposes.

Key identity: softmax is invariant to a per-row scale, so divide the whole
row by exp(s_r) and fold exp(d_j) into the stationary Wh.  The remaining
per-element factor is

    p[j,r] = exp(leaky_relu(t) - t) * m[j,r]     (t = s_r + d_j)
           = max(1, exp(-0.8 t)) * m[j,r]

computed two ways, assigned per key-chunk to balance ScalarE vs VectorE:
  path A (VectorE only): q = (zb[r] * w[j]) max 1   (one fused tensor_scalar,
      zb = exp(-0.8 s) broadcast, w = exp(-0.8 d) per-partition);
      p = q * mask01                               (grouped tensor_tensor)
  path C (ScalarE + VectorE): the host ships in_c = madd - 0.8(s+d) with
      madd the 0/-1000 additive mask, so E = Exp(in_c) = q*m needs NO bias
      and groups over 8 chunks in ONE ScalarE op; then
      p = E max mask01 (grouped tensor_tensor) -- masked entries stay 0,
      unmasked become max(q, 1).

softmax denominators ride along as the exp(d) column in the stationary
operand; division + ELU happen on the host on the tiny per-head
[HID+1, 512] outputs.

Wh = x @ W (0.4%% of the FLOPs) plus the per-node attention vectors
f_src/f_dst are computed on the host in fp32 and shipped pre-rounded to
bf16; all on-device attention/aggregation math runs in bf16 with fp32 PSUM
accumulation.
"""

import os
import sys
import time
from contextlib import ExitStack

for _p in ("/opt/trn_rl_repo", "/root/.axon_site/_ro/trn_rl_repo"):
    if os.path.isdir(_p) and _p not in sys.path:
        sys.path.append(_p)

import numpy as np
import ml_dtypes

import bass_rust
import concourse.bass as bass
import concourse.tile as tile
from concourse import mybir
from concourse.bass_utils import run_bass_kernel_spmd

BF16 = ml_dtypes.bfloat16
F32 = mybir.dt.float32
BF = mybir.dt.bfloat16

N = 4096          # nodes
NCORES = 8
R = N // NCORES   # rows (queries) per core
CJ = N // 128     # 32 key chunks
FIN = 512         # input feature dim of both layers
NA = 8            # path-A chunks per head (chunks 0..NA-1); NC = CJ - NA
GRPA = 4          # A-path group size
GRPC = 8          # C-path group size (= mask slice size)
NMQ = 4           # mask/whx DMA slices
SLC = CJ // NMQ   # 8 chunks per slice

CORE_IDS = list(range(NCORES))

LAST_PERF = {}


# ---------------------------------------------------------------------------
# walrus workaround: it rejects instructions carrying >1 sync-wait command
# ("Too many sync wait commands").  Move excess waits onto preceding
# same-engine NoOps -- semantically identical (same-engine waits are totally
# ordered before the instruction).
def _split_excess_waits(nc, max_waits: int = 1) -> int:
    n_split = 0
    for fn in nc.m.functions:
        for bb in fn.blocks:
            insts = bb.instructions
            new_insts = []
            changed = False
            for ins in insts:
                si = ins.sync_info
                waits = list(si.on_wait) if si is not None else []
                if len(waits) > max_waits:
                    extra, keep = waits[:-max_waits], waits[-max_waits:]
                    for k in range(0, len(extra), max_waits):
                        chunk = extra[k : k + max_waits]
                        nop = bass_rust.InstNoOp(
                            name=f"{ins.name}-wsplit{k}", ins=[], outs=[]
                        )
                        nop.engine = ins.engine
                        nop.sync_info = mybir.SyncInfo(on_wait=chunk, on_update=[])
                        new_insts.append(nop)
                        n_split += 1
                    si.on_wait = keep
                    changed = True
                new_insts.append(ins)
            if changed:
                bb.instructions = new_insts
    return n_split


# ---------------------------------------------------------------------------
def _build_layer(H: int, HID: int, na: int = NA):
    """One GAT layer, per-core program.

    Inputs (per core):
      whxin  [128, CJ, H, WPH] bf16  Wh*exp(d) per head + exp(d) column
      maskM  [128, CJ, R]   bf16  multiplicative 0/1 adjacency, transposed
      zbB    [128, H, R]    bf16  exp(-0.8 f_src) of this core's rows (bcast)
      wcol   [128, H*CJ]    f32   [p, h*CJ+c] = exp(-0.8 f_dst[h, 128c+p])
      incM   [128, H, NCG, GRPC, R] bf16  madd - 0.8(s+d) for C-path chunks
    Output:
      agg    [H, HID+1, R]  f32   rows 0..HID-1: unnormalized att @ Wh
                                  (transposed); row HID: softmax denominator
    """
    WPH = HID + 2  # per-head stride in Whx: HID cols + exp(d) col + pad
    nc_chunks = CJ - na
    NCG = nc_chunks // GRPC  # C-path groups per head

    nc = bass.Bass("TRN2", debug=False, num_devices=NCORES)
    whxin = nc.dram_tensor("whxin", [128, CJ, H, WPH], BF, kind="ExternalInput")
    maskM = nc.dram_tensor("maskM", [128, CJ, R], BF, kind="ExternalInput")
    zbB = nc.dram_tensor("zbB", [128, H, R], BF, kind="ExternalInput")
    wcol = nc.dram_tensor("wcol", [128, H * CJ], F32, kind="ExternalInput")
    incM = nc.dram_tensor(
        "incM", [128, H, NCG, GRPC, R], BF, kind="ExternalInput"
    )
    agg = nc.dram_tensor("agg", [H, HID + 1, R], F32, kind="ExternalOutput")

    EXP = mybir.ActivationFunctionType.Exp
    MAX = mybir.AluOpType.max
    MUL = mybir.AluOpType.mult

    with tile.TileContext(nc) as tc, ExitStack() as ctx:
        cpool = ctx.enter_context(tc.tile_pool(name="const", bufs=1))
        ipool = ctx.enter_context(tc.tile_pool(name="inc", bufs=8))
        tpool = ctx.enter_context(tc.tile_pool(name="work", bufs=3))
        qpool = ctx.enter_context(tc.tile_pool(name="qwork", bufs=2))
        opool = ctx.enter_context(tc.tile_pool(name="out", bufs=2))
        paq = ctx.enter_context(tc.tile_pool(name="psa", bufs=3, space="PSUM"))

        # in_c prefetch: head 0's group DMAs go on the Activation hwdge
        # queue -- a second DMA stream running in parallel with the SP
        # queue's mask/whx while ScalarE is still idle.  Later heads stream
        # on SP in compute order (ipool bufs bound the run-ahead).
        inc_t = {}
        def fetch_inc(h, g, eng=None):
            t = ipool.tile([128, GRPC, R], BF, tag="inc")
            (eng or nc.sync).dma_start(t[:], incM[:, h, g])
            inc_t[(h, g)] = t

        mask_t = [
            cpool.tile([128, SLC, R], BF, tag=f"mask{m}", name=f"mask{m}")
            for m in range(NMQ)
        ]
        whx_t = [
            cpool.tile([128, SLC, H, WPH], BF, tag=f"whx{m}", name=f"whx{m}")
            for m in range(NMQ)
        ]
        hs = SLC // 2
        # heads 0+1 in_c on the parallel Activation queue (ScalarE is idle
        # during startup)
        for hh in range(min(2, H)):
            for g in range(NCG):
                fetch_inc(hh, g, eng=nc.scalar)
        # SP queue, ordered for the C0 chain (m1) and A0 (m0a/w/zb)
        nc.sync.dma_start(mask_t[0][:, 0:hs, :], maskM[:, 0:hs, :])
        nc.sync.dma_start(whx_t[0][:, 0:hs], whxin[:, 0:hs])
        w_t = cpool.tile([128, H * CJ], F32, tag="wcol")
        nc.sync.dma_start(w_t[:], wcol[:])
        zb_t = cpool.tile([128, H, R], BF, tag="zb")
        nc.sync.dma_start(zb_t[:], zbB[:])
        nc.sync.dma_start(mask_t[0][:, hs:SLC, :], maskM[:, hs:SLC, :])
        nc.sync.dma_start(whx_t[0][:, hs:SLC], whxin[:, hs:SLC])
        for mq in range(1, NMQ):
            cs = slice(mq * SLC, (mq + 1) * SLC)
            nc.sync.dma_start(mask_t[mq][:], maskM[:, cs, :])
            nc.sync.dma_start(whx_t[mq][:], whxin[:, cs])
        if H > 2:
            for g in range(NCG):
                fetch_inc(2, g)

        def mask_ap(c0, G):  # [128, G, R] view; groups never straddle slices
            return mask_t[c0 // SLC][:, c0 % SLC : c0 % SLC + G, :]

        def whx_ap(c, h):  # [128, HID+1] stationary for chunk c, head h
            return whx_t[c // SLC][:, c % SLC, h, 0 : HID + 1]

        # per-head chunk-group schedule: the A group (fed by the earliest
        # DMAs) first, then the C groups
        agrps = [(c, min(GRPA, na - c), "a") for c in range(0, na, GRPA)]
        cgrps = [(na + g * GRPC, GRPC, "c") for g in range(NCG)]
        groups = agrps + cgrps

        # ---- attention + aggregation --------------------------------------
        # the PSUM->SBUF copy of head h is emitted AFTER head h+1's groups:
        # it depends on head h's last matmul, and emitting it eagerly would
        # head-of-line-block the next head's activations on the in-order
        # ScalarE queue while PE drains.
        def flush_prev(prev):
            ph, ppa = prev
            o = opool.tile([HID + 1, R], F32, tag="aggo", name="aggo")
            nc.scalar.copy(o[:], ppa[:])
            nc.sync.dma_start(agg[ph], o[:])

        prev = None
        for h in range(H):
            pa = paq.tile([HID + 1, R], F32, tag="psa")
            for gi, (c0, G, kind) in enumerate(groups):
                if kind == "a":
                    # path A: q = (zb * w_j) max 1 ; p = q * m01   (all V)
                    p3p = tpool.tile([128, GRPA, R], BF, tag="p3a")
                    qp = qpool.tile([128, GRPA, R], BF, tag="qa")
                    for k in range(G):
                        o_ix = h * CJ + c0 + k
                        nc.vector.tensor_scalar(
                            qp[:, k, :], zb_t[:, h, :],
                            w_t[:, o_ix : o_ix + 1], 1.0, op0=MUL, op1=MAX,
                        )
                    nc.vector.tensor_tensor(
                        p3p[:, 0:G, :], qp[:, 0:G, :], mask_ap(c0, G), op=MUL,
                    )
                else:
                    # path C: E = exp(in_c) = q*m (one grouped ScalarE op),
                    # p = E max m01 (one grouped V op)
                    g = (c0 - na) // GRPC
                    p3p = tpool.tile([128, GRPC, R], BF, tag="p3c")
                    qp = qpool.tile([128, GRPC, R], BF, tag="qc")
                    it = inc_t.pop((h, g))
                    nc.scalar.activation(qp[:, 0:G, :], it[:], EXP)
                    if h >= 1 and h + 2 < H:
                        fetch_inc(h + 2, g)
                    nc.vector.tensor_tensor(
                        p3p[:, 0:G, :], qp[:, 0:G, :], mask_ap(c0, G), op=MAX,
                    )
                for k in range(G):
                    nc.tensor.matmul(
                        pa[:], whx_ap(c0 + k, h), p3p[:, k, :],
                        start=(gi == 0 and k == 0),
                        stop=(gi == len(groups) - 1 and k == G - 1),
                    )
            if prev is not None:
                flush_prev(prev)
            prev = (h, pa)
        flush_prev(prev)

    return nc


_PROGS = {}


def _get_prog(H, HID, na=NA):
    """Build (and cache) the layer program with the walrus wait-split fix
    applied.  The fix is HW-only: CoreSim's event loop rejects the injected
    NoOps, so sim users should call _build_layer directly."""
    key = (H, HID, na)
    if key not in _PROGS:
        nc = _build_layer(H, HID, na)
        _split_excess_waits(nc)
        _PROGS[key] = nc
    return _PROGS[key]


def _elu(v):
    return np.where(v > 0, v, np.expm1(np.minimum(v, 0.0))).astype(np.float32)


def _host_inputs(f_src, f_dst, adj, Wh, H, na=NA):
    """Shared per-layer host prep.  f_src/f_dst [N, H] f32, adj [N, N] i32,
    Wh [N, H*HID] f32 (pre-activation per-head features)."""
    HID = Wh.shape[1] // H
    WPH = HID + 2
    NCG = (CJ - na) // GRPC
    fdst_arr = np.ascontiguousarray(
        f_dst.T.reshape(H, CJ, 128).transpose(2, 0, 1).reshape(128, H * CJ)
    ).astype(np.float32)
    w_arr = np.exp(-0.8 * fdst_arr).astype(np.float32)   # exp(-0.8 f_dst)

    # exp(f_dst) folded into the stationary operand; ones-col becomes exp(d)
    ev = np.exp(f_dst).astype(np.float32)  # [N, H]
    whx = np.zeros((128, CJ, H, WPH), np.float32)
    whx[:, :, :, :HID] = (
        (Wh.reshape(N, H, HID) * ev[:, :, None])
        .reshape(CJ, 128, H, HID).transpose(1, 0, 2, 3)
    )
    whx[:, :, :, HID] = ev.reshape(CJ, 128, H).transpose(1, 0, 2)

    # -0.8*f_dst laid out [128, c, h] for the C-path combined tiles
    fd8 = (
        (-0.8 * f_dst).T.reshape(H, CJ, 128).transpose(2, 1, 0)
    ).astype(np.float32)  # [128, CJ, H]

    shared = {
        "wcol": w_arr,
        "whxin": whx.astype(BF16),
    }
    per_core = []
    for i in range(NCORES):
        rows = slice(R * i, R * (i + 1))
        adjT = adj[rows, :].T.astype(np.float32)  # [N, R] 0/1
        madd = np.ascontiguousarray(
            ((adjT - 1.0) * 1000.0).reshape(CJ, 128, R).transpose(1, 0, 2)
        )  # [128, CJ, R] additive 0/-1000
        fs = np.ascontiguousarray(f_src[rows, :].T)  # [H, R]
        zln = -0.8 * fs
        d = dict(shared)
        d["maskM"] = np.ascontiguousarray(
            adjT.reshape(CJ, 128, R).transpose(1, 0, 2)
        ).astype(BF16)
        d["zbB"] = np.broadcast_to(
            np.exp(zln)[None, :, :], (128, H, R)
        ).astype(BF16)
        # in_c[p, h, g, k, r] = madd[p, na+g*GRPC+k, r] + zln[h, r] + fd8[p, c, h]
        cc = madd[:, na:, :].reshape(128, NCG, GRPC, 1, R)  # broadcast over h
        inc = (
            cc
            + zln[None, None, None, :, :]
            + fd8[:, na:, :].reshape(128, NCG, GRPC, H, 1)
        )  # [128, NCG, GRPC, H, R]
        d["incM"] = np.ascontiguousarray(
            inc.transpose(0, 3, 1, 2, 4)
        ).astype(BF16)
        per_core.append(d)
    return per_core


def _run_layer(nc, in_maps, H, HID, tag):
    t0 = time.time()
    res = run_bass_kernel_spmd(nc, in_maps, core_ids=CORE_IDS)
    LAST_PERF[f"{tag}_wall_s"] = time.time() - t0
    LAST_PERF[f"{tag}_exec_ns"] = res.exec_time_ns

    hT = np.empty((H * HID, N), np.float32)
    for i in range(NCORES):
        a = res.results[i]["agg"]  # [H, HID+1, R]
        denom = a[:, HID : HID + 1, :]
        hT[:, R * i : R * (i + 1)] = (a[:, :HID, :] / denom).reshape(H * HID, R)
    return hT


def kernel(x, adj, W1, a1, W2, a2):
    x = np.asarray(x, np.float32)
    adj = np.asarray(adj, np.int32)
    W1 = np.asarray(W1, np.float32)
    a1 = np.asarray(a1, np.float32)
    W2 = np.asarray(W2, np.float32)
    a2 = np.asarray(a2, np.float32)

    H1, HID1, OUT = W1.shape[0], W1.shape[2], W2.shape[1]

    progA = _get_prog(H1, HID1)
    progB = _get_prog(1, OUT)

    # ---- layer 1 ----------------------------------------------------------
    W1c = np.ascontiguousarray(W1.transpose(1, 0, 2).reshape(FIN, H1 * HID1))
    wsrc1 = np.einsum("hfk,hk->fh", W1, a1[:, :HID1, 0]).astype(np.float32)
    wdst1 = np.einsum("hfk,hk->fh", W1, a1[:, HID1:, 0]).astype(np.float32)
    f_src1 = x @ wsrc1  # [N, H]
    f_dst1 = x @ wdst1
    Wh1 = x @ W1c  # [N, H1*HID1]

    in_maps = _host_inputs(f_src1, f_dst1, adj, Wh1, H1)
    hT = _run_layer(progA, in_maps, H1, HID1, "layer1")
    hcatT = _elu(hT)  # [512, N] == h_cat.T (concat=True applies elu)

    # ---- layer 2 ----------------------------------------------------------
    hcat = np.ascontiguousarray(hcatT.T)  # [N, 512]
    wsrc2 = (W2 @ a2[:OUT, 0]).astype(np.float32)[:, None]
    wdst2 = (W2 @ a2[OUT:, 0]).astype(np.float32)[:, None]
    f_src2 = hcat @ wsrc2  # [N, 1]
    f_dst2 = hcat @ wdst2
    Wh2 = hcat @ W2  # [N, OUT]
    in_maps2 = _host_inputs(f_src2, f_dst2, adj, Wh2, 1)
    outT = _run_layer(progB, in_maps2, 1, OUT, "layer2")
    # layer 2: concat=False -> no inner elu; final output = elu(out)
    return np.ascontiguousarray(_elu(outT).T)
